# revision 8
# baseline (speedup 1.0000x reference)
"""GPT-OSS MoE experts kernel for Trainium2 (8 NeuronCores, expert-parallel).

Strategy (v2)
-------------
- Expert-parallel: core e owns expert e's weights (1/8 of total weight bytes,
  read exactly once -> memory-bound). Host does routing (gather tokens per
  expert), weight re-staging, and the final scatter-add combine.
- The reference's per-32-block fp8 quant-dequant collapses exactly to
  "round each element to 4 significant bits (RTNE)" (block scale is a power
  of two; the +-448 clip never binds). On device this is 3 VectorE ops; the
  4-significant-bit activation values are then EXACT in fp16.
- gate_up_proj stays fp16 (quantizing layer 1 trips the inter-activation
  fp8 re-quantization boundaries: measured 2.9e-2 absmax-rel vs the 2e-2
  budget). down_proj + its bias are stored INT8 with one power-free scale
  per expert (measured 1.38e-2), cast int8->fp16 in-flight by SWDGE DMA;
  the scale is folded into the routing-weight multiply. Halves layer-2
  weight traffic of this DMA-bound kernel.
- Padding trimmed: contraction is 2881 rows (2880 + ones/bias row) = 22 full
  128-row tiles + one 65-row tail; output dims are 2880 = 22 full 128-wide
  tiles + one 64-wide. ~4% fewer bytes than padding everything to 2944.
- Form-B matmuls: weight tiles are the STATIONARY operand, all tokens ride
  the moving free dim (ccap <= 248). Outputs land output-major, feeding
  layer 2 directly with no on-chip transposes. Biases ride as an extra
  contraction row against a constant-1 activation row.
- Layer 2 is PIPELINED into layer 1: its contraction is split into groups
  G0 = [tail, i-tiles 0..12], G1 = [13..19], G2 = [20, 21]. As soon as a
  group's inter tiles exist, all 23 h-tiles accumulate that group in
  sub-bank PSUM accumulators (3 per 2KB bank, batches of 9 h-tiles) and
  spill/add into an SBUF f32 ysum. This keeps the PE busy through the
  layer-1 -> layer-2 transition and leaves only ~23x2 matmuls plus the
  combine after the last layer-1 slab arrives.
- Slab for n-tile 21 is fetched early so the last arrivals feed minimal
  trailing compute; w1 weight DMAs ride the Sync HWDGE ring, int8 w2 rides
  the GpSimd SWDGE ring, activations/outputs ride the Scalar ring.
"""

import functools
import sys

sys.path.insert(0, "/opt/trn_rl_repo")

import numpy as np

import concourse.bass as bass  # noqa: F401
import concourse.mybir as mybir
import concourse.tile as tile
from concourse import bacc
from concourse.bass_utils import run_bass_kernel_spmd

P = 128
H = 2880          # hidden dim == intermediate dim
NE = 8            # experts == cores
KTF = 22          # full 128-row contraction tiles (rows 0..2815)
TR = 65           # tail rows: 2816..2879 + ones/bias row (2880)
NT = 23           # output n-tiles per 2880-wide dim (22 full + one 64-wide)
VC = float(2 ** 20 + 1)   # Veltkamp constant: RTNE to 4 significant bits
MAXTOK = 248              # per-pass token capacity (2 accs fit one PSUM bank)
# layer-2 contraction groups (main i-tiles; the 65-row tail rides with GK[0]):
GK = [list(range(0, 5)), list(range(5, 11)), list(range(11, 19)), list(range(19, 22))]
# block-emission slot (layer-1 iter idx) per (group, batch): group g batch b
# runs after iter GEMIT[g] + b
GEMIT = [6, 12, 19, None]  # None: after the whole layer-1 loop

f32 = mybir.dt.float32
f16 = mybir.dt.float16
i8 = mybir.dt.int8
AF = mybir.ActivationFunctionType
ALU = mybir.AluOpType


def _rtne4(x):
    """Round f32 elements to 4 significant bits, RTNE (== reference
    quant_dequant_fp8 up to e4m3-subnormal leftovers)."""
    c = np.float32(VC)
    t = (x * c).astype(np.float32)
    return (t - (t - x)).astype(np.float32)


@functools.lru_cache(maxsize=4)
def _build(ccap):
    """Per-core Bass program; ccap = padded token capacity (<= MAXTOK)."""
    nc = bacc.Bacc(None, target_bir_lowering=False)

    xt_d = nc.declare_dram_parameter("xt", [P, NT, ccap], f16, isOutput=False)
    w1m_d = nc.declare_dram_parameter("w1m", [KTF, P, KTF, 2 * P], f16, isOutput=False)
    w1m22_d = nc.declare_dram_parameter("w1m22", [P, KTF, P], f16, isOutput=False)
    w1t_d = nc.declare_dram_parameter("w1t", [TR, NT, 2 * P], f16, isOutput=False)
    w2g_d = [
        nc.declare_dram_parameter(f"w2g{g}", [P, NT, len(GK[g]), P], i8, isOutput=False)
        for g in range(len(GK))
    ]
    w2t_d = nc.declare_dram_parameter("w2t", [TR, NT, P], i8, isOutput=False)
    wr_d = nc.declare_dram_parameter("wr", [P, ccap], f32, isOutput=False)
    y_d = nc.declare_dram_parameter("y", [NT, P, ccap], f16, isOutput=True)

    apb = 512 // ccap           # f32 accumulators per 2KB PSUM bank
    bsz = 3 * apb               # h-tiles per layer-2 batch (3 banks per block)
    batches = [(s, min(s + bsz, NT)) for s in range(0, NT, bsz)]
    nb = len(batches)
    order = [22] + list(range(22))   # layer-1 iteration order (n-tile ids)

    gbufs = [2, 1, 2, 2]   # chunk double-buffering per layer-2 group

    with tile.TileContext(nc) as tc:
        with (
            tc.tile_pool(name="consts", bufs=1) as consts,
            tc.tile_pool(name="w1p", bufs=5) as w1p,
            tc.tile_pool(name="w2p", bufs=1) as w2p,
            tc.tile_pool(name="tmp", bufs=2) as tmp,
            tc.tile_pool(name="psum", bufs=2, space="PSUM") as psum,
        ):
            # resident tensors
            xts = consts.tile([P, NT, ccap], f16, tag="xt", name="xts")
            nc.sync.dma_start(xts, xt_d[:])
            w1t_s = consts.tile([TR, NT, 2 * P], f16, tag="w1t", name="w1t_s")
            w2t_s = consts.tile([TR, NT, P], f16, tag="w2t", name="w2t_s")
            wrep = consts.tile([P, ccap], f32, tag="wrep", name="wrep")
            nc.scalar.dma_start(wrep, wr_d[:])
            interT = consts.tile([P, NT, ccap], f16, tag="interT", name="interT")
            nc.vector.memset(interT[64:65, 22, :], 1.0)   # layer-2 bias row
            ysum = consts.tile([P, NT, ccap], f32, tag="ysum", name="ysum")

            # HAM warmup while xt + the first slab stream in
            wtile = consts.tile([P, P], f16, tag="wtile", name="wtile")
            nc.vector.memset(wtile, 0.25)
            warm = psum.tile([P, 512], f32, tag="gu", name="warm")
            for _ in range(64):
                nc.tensor.matmul(warm[:, :P], wtile, wtile,
                                 start=True, stop=True, skip_group_check=True)

            # gate the Pool(SWDGE) ring behind early layer-1 progress so the
            # int8 w2 stream does not steal startup bandwidth from xt/w1
            gdum = consts.tile([1, 1], f16, tag="gdum", name="gdum")
            nc.gpsimd.tensor_copy(gdum, interT[0:1, 0, 0:1])
            nc.gpsimd.dma_start(w2t_s, w2t_d[:])      # cast int8 -> fp16

            def load_w2chunk(gi, b):
                ht0, ht1 = batches[b]
                t = w2p.tile([P, bsz, len(GK[gi]), P], f16, tag=f"w2c{gi}",
                             name=f"w2c{gi}", bufs=gbufs[gi])
                nc.gpsimd.dma_start(t[:, : ht1 - ht0], w2g_d[gi][:, ht0:ht1])
                return t

            def l2_block(gi, b):
                ht0, ht1 = batches[b]
                chunk = load_w2chunk(gi, b)
                kts = GK[gi]
                pieces = (["tail"] + kts) if gi == 0 else kts
                last = len(GK) - 1
                blk = psum.tile([P, 3, 512], f32, tag="accblk", name="accblk")
                ysb = None
                if gi == last:
                    ysb = tmp.tile([P, bsz, ccap], f16, tag="ysb", name="ysb")
                npc = len(pieces)
                for j, ht in enumerate(range(ht0, ht1)):
                    W2 = 64 if ht == 22 else P
                    c0 = (j % apb) * ccap
                    acc = blk[:W2, j // apb, c0 : c0 + ccap]
                    for pi, pc in enumerate(pieces):
                        st, sp = pi == 0, pi == npc - 1
                        if pc == "tail":
                            nc.tensor.matmul(acc, w2t_s[:, ht, :W2],
                                             interT[:TR, 22, :], start=st, stop=sp)
                        else:
                            nc.tensor.matmul(acc, chunk[:, j, kts.index(pc), :W2],
                                             interT[:, pc, :], start=st, stop=sp)
                    if gi == 0:
                        nc.vector.tensor_copy(ysum[:W2, ht, :], acc)
                    elif gi < last:
                        nc.vector.tensor_add(ysum[:W2, ht, :], ysum[:W2, ht, :], acc)
                    else:
                        tvv = tmp.tile([P, ccap], f32, tag="tvv", name="tvv")
                        nc.vector.tensor_add(tvv[:W2], ysum[:W2, ht, :], acc)
                        nc.vector.tensor_mul(ysb[:W2, j, :], tvv[:W2], wrep[:W2])
                if gi == last:
                    nc.scalar.dma_start(
                        y_d[ht0:ht1].rearrange("t p c -> p t c"),
                        ysb[:, : ht1 - ht0, :],
                    )

            slab21 = None
            for idx, nt in enumerate(order):
                W = 64 if nt == 22 else P
                if nt == 22:
                    slab = w1p.tile([P, KTF, P], f16, tag="slab", name="slab")
                    nc.sync.dma_start(slab, w1m22_d[:])
                    nc.sync.dma_start(w1t_s, w1t_d[:])
                elif nt == 21:
                    slab = slab21
                else:
                    slab = w1p.tile([P, KTF, 2 * P], f16, tag="slab", name="slab")
                    nc.sync.dma_start(slab, w1m_d[nt])

                # one open accumulation group per PSUM bank at a time: all
                # gate matmuls (full group), then all up matmuls
                gu = psum.tile([P, 512], f32, tag="gu", name="gu")
                for kt in range(KTF):
                    nc.tensor.matmul(gu[:W, :ccap], slab[:, kt, 0:W],
                                     xts[:, kt, :], start=(kt == 0), stop=False)
                nc.tensor.matmul(gu[:W, :ccap], w1t_s[:, nt, 0:W],
                                 xts[:TR, 22, :], start=False, stop=True)
                for kt in range(KTF):
                    nc.tensor.matmul(gu[:W, 256 : 256 + ccap], slab[:, kt, W : 2 * W],
                                     xts[:, kt, :], start=(kt == 0), stop=False)
                nc.tensor.matmul(gu[:W, 256 : 256 + ccap], w1t_s[:, nt, W : 2 * W],
                                 xts[:TR, 22, :], start=False, stop=True)

                # swiglu: gate=min(G,7); up1=clip(U,-7,7)+1; x=gate*sig(1.702g)*up1
                gate = tmp.tile([P, ccap], f32, tag="t_gate", name="t_gate")
                nc.vector.tensor_scalar_min(gate[:W], gu[:W, :ccap], 7.0)
                sig = tmp.tile([P, ccap], f32, tag="t_sig", name="t_sig")
                nc.scalar.activation(sig[:W], gate[:W], AF.Sigmoid, scale=1.702)
                up1 = tmp.tile([P, ccap], f32, tag="t_up", name="t_up")
                nc.vector.tensor_scalar(up1[:W], gu[:W, 256 : 256 + ccap],
                                        1.0, -6.0, ALU.add, ALU.max)
                nc.vector.tensor_scalar_min(up1[:W], up1[:W], 8.0)
                nc.vector.tensor_mul(gate[:W], gate[:W], sig[:W])
                xv = tmp.tile([P, ccap], f32, tag="t_xv", name="t_xv")
                nc.vector.tensor_mul(xv[:W], gate[:W], up1[:W])
                tv = tmp.tile([P, ccap], f32, tag="t_tv", name="t_tv")
                nc.vector.tensor_scalar_mul(tv[:W], xv[:W], VC)
                nc.vector.tensor_sub(xv[:W], tv[:W], xv[:W])
                nc.vector.tensor_sub(interT[:W, nt, :], tv[:W], xv[:W])

                if idx == 15:
                    # early-fetch the last slab so the final layer-1 iteration
                    # is not gated on the last weight DMA of the kernel (the
                    # DMA bubble lands where layer-2 compute hides it)
                    slab21 = w1p.tile([P, KTF, 2 * P], f16, tag="slab21",
                                      name="slab21", bufs=1)
                    nc.sync.dma_start(slab21, w1m_d[21])
                for gi, e0 in enumerate(GEMIT):
                    if e0 is not None and e0 <= idx < e0 + nb:
                        l2_block(gi, idx - e0)

            for b in range(nb):
                l2_block(len(GK) - 1, b)

    nc.finalize()
    return nc


def _stage_weights(gup, gub, dn, dnb, e):
    """Re-stage one expert's weights into the device layouts."""
    Gt = np.ascontiguousarray(gup[e, 0::2, :].T).astype(np.float16)  # [h, i]
    Ut = np.ascontiguousarray(gup[e, 1::2, :].T).astype(np.float16)
    Gm = Gt[: KTF * P].reshape(KTF, P, H)
    Um = Ut[: KTF * P].reshape(KTF, P, H)

    w1m = np.empty((KTF, P, KTF, 2 * P), np.float16)
    w1m[:, :, :, 0:P] = Gm[:, :, : KTF * P].reshape(KTF, P, KTF, P).transpose(2, 1, 0, 3)
    w1m[:, :, :, P : 2 * P] = Um[:, :, : KTF * P].reshape(KTF, P, KTF, P).transpose(2, 1, 0, 3)
    w1m22 = np.empty((P, KTF, P), np.float16)
    w1m22[:, :, 0:64] = Gm[:, :, KTF * P :].transpose(1, 0, 2)
    w1m22[:, :, 64:128] = Um[:, :, KTF * P :].transpose(1, 0, 2)

    Gtl = np.vstack([Gt[KTF * P :], gub[e, 0::2].astype(np.float16)[None]])  # [65, i]
    Utl = np.vstack([Ut[KTF * P :], gub[e, 1::2].astype(np.float16)[None]])
    w1t = np.empty((TR, NT, 2 * P), np.float16)
    w1t[:, :KTF, 0:P] = Gtl[:, : KTF * P].reshape(TR, KTF, P)
    w1t[:, :KTF, P : 2 * P] = Utl[:, : KTF * P].reshape(TR, KTF, P)
    w1t[:, 22, 0:64] = Gtl[:, KTF * P :]
    w1t[:, 22, 64:128] = Utl[:, KTF * P :]

    Dt = np.ascontiguousarray(dn[e].T)          # [i, h] f32
    s = max(np.abs(Dt).max(), np.abs(dnb[e]).max()) / 127.0
    Q = np.round(np.vstack([Dt, dnb[e][None]]) / s)
    Q = np.clip(Q, -127, 127).astype(np.int8)   # [2881, h]
    Qm = Q[: KTF * P].reshape(KTF, P, H)

    def group(kts):
        A = np.zeros((P, NT, len(kts), P), np.int8)
        sel = Qm[kts]                            # [kg, P, h]
        A[:, :KTF] = sel[:, :, : KTF * P].reshape(len(kts), P, KTF, P).transpose(1, 2, 0, 3)
        A[:, 22, :, 0:64] = sel[:, :, KTF * P :].transpose(1, 0, 2)
        return A

    w2t = np.zeros((TR, NT, P), np.int8)
    Qt = Q[KTF * P :]                            # [65, h]
    w2t[:, :KTF] = Qt[:, : KTF * P].reshape(TR, KTF, P)
    w2t[:, 22, 0:64] = Qt[:, KTF * P :]

    out = dict(w1m=w1m, w1m22=w1m22, w1t=w1t, w2t=w2t)
    for g, kts in enumerate(GK):
        out[f"w2g{g}"] = group(kts)
    return out, s


def _stage(inputs):
    """Host-side routing + weight re-staging. Returns (nc, passes, assigns, T)."""
    hs = np.ascontiguousarray(np.asarray(inputs["hidden_states"], dtype=np.float32))
    ri = np.asarray(inputs["router_indices"]).astype(np.int64)
    rw = np.asarray(inputs["routing_weights"], dtype=np.float32)
    gup = np.asarray(inputs["gate_up_proj"], dtype=np.float32)
    gub = np.asarray(inputs["gate_up_proj_bias"], dtype=np.float32)
    dn = np.asarray(inputs["down_proj"], dtype=np.float32)
    dnb = np.asarray(inputs["down_proj_bias"], dtype=np.float32)

    T = hs.shape[0]
    topk = ri.shape[1]

    flat_e = ri.reshape(-1)
    order = np.argsort(flat_e, kind="stable")
    counts = np.bincount(flat_e, minlength=NE)
    starts = np.zeros(NE + 1, np.int64)
    starts[1:] = np.cumsum(counts)
    maxc = int(counts.max())
    npass = max(1, -(-maxc // MAXTOK))
    percap = -(-maxc // npass)
    ccap = max(32, -(-percap // 16) * 16)

    x_dq = _rtne4(hs).astype(np.float16)   # 4-sig-bit values: exact in fp16
    rw_flat = rw.reshape(-1)

    weights, scales = [], []
    for e in range(NE):
        w, s = _stage_weights(gup, gub, dn, dnb, e)
        weights.append(w)
        scales.append(s)

    passes, assigns = [], []
    for p in range(npass):
        in_maps, passigns = [], []
        for e in range(NE):
            a_all = order[starts[e] : starts[e + 1]]
            a = a_all[p * ccap : (p + 1) * ccap]
            toks = a // topk
            ce = len(a)
            passigns.append((a, toks))

            xx = np.zeros((NT * P, ccap), np.float16)
            xx[:H, :ce] = x_dq[toks].T
            xx[H, :] = np.float16(1.0)            # ones row for the biases
            xt = np.ascontiguousarray(xx.reshape(NT, P, ccap).transpose(1, 0, 2))

            wr_rep = np.zeros((P, ccap), np.float32)
            wr_rep[:, :ce] = (rw_flat[a] * scales[e])[None, :]

            m = dict(weights[e])
            m.update(xt=xt, wr=wr_rep)
            in_maps.append(m)
        passes.append(in_maps)
        assigns.append(passigns)

    nc = _build(ccap)
    return nc, passes, assigns, T


def kernel(**inputs):
    nc, passes, assigns, T = _stage(inputs)
    out = np.zeros((T, H), np.float32)
    for in_maps, passigns in zip(passes, assigns):
        res = run_bass_kernel_spmd(nc, in_maps, list(range(NE)))
        for e in range(NE):
            a, toks = passigns[e]
            if len(a):
                yt = res.results[e]["y"].reshape(NT * P, -1)[:H, : len(a)]
                np.add.at(out, toks, yt.T.astype(np.float32))
    return out
